# revision 4
# baseline (speedup 1.0000x reference)
"""Bidirectional GRU (T=512, B=32, I=H=512) on 8 Trainium2 NeuronCores.

Sharding: core c -> (direction d = c//4, batch slice j = c%4 of 8).
Backward direction is handled by feeding that core a time-reversed input
and un-reversing its output on the host, so all 8 cores run the same
SPMD program.

Per-core device program:
  Phase 1: xg = x @ w_ih^T + biases as one big fp16 GEMM at full PE
           utilization; xg kept SBUF-resident in fp16 (12 MB).
  Phase 2: 512 sequential GRU steps. Weight-stationary matmuls
           (gate-dim on partitions) so the per-step elementwise work is
           (128, 32)-shaped; h state ping-pongs through 4 fixed SBUF
           slots; per-step y written out by DMA.

Numerics: fp16 operands with fp32 PSUM accumulation and fp32 gate math;
measured end-to-end absmax error vs the fp32 reference ~2.3e-3 on
y (|y|max ~4.1), i.e. ~6e-4 scale-relative.
"""

import numpy as np

T, B, I, H = 512, 32, 512, 512
NB = 8          # batch per core
KC = 4          # contraction chunks (512/128)
GC = 12         # gate-dim chunks (1536/128)
N_CORES = 8
UNROLL = 8
P1_BLOCK = 512  # phase-1 moving-operand block


def build(t_steps=T, unroll=UNROLL):
    import concourse.bacc as bacc
    import concourse.bass as bass
    import concourse.mybir as mybir
    import concourse.tile as tile
    from contextlib import ExitStack

    f16, f32 = mybir.dt.float16, mybir.dt.float32
    TBS = t_steps * NB

    nc = bacc.Bacc("TRN2", target_bir_lowering=False, debug=False,
                   num_devices=N_CORES)

    xT = nc.dram_tensor("xT", [128, KC, TBS], f16, kind="ExternalInput").ap()
    wih_d = nc.dram_tensor("wih", [128, KC, GC, 128], f16, kind="ExternalInput").ap()
    whh_d = nc.dram_tensor("whh", [128, KC, GC, 128], f16, kind="ExternalInput").ap()
    biasg_d = nc.dram_tensor("biasg", [128, GC], f32, kind="ExternalInput").ap()
    bhhn_d = nc.dram_tensor("bhhn", [128, KC, NB], f32, kind="ExternalInput").ap()
    h0t_d = nc.dram_tensor("h0t", [128, KC, NB], f16, kind="ExternalInput").ap()
    y_d = nc.dram_tensor("y", [t_steps, 128, KC, NB], f16, kind="ExternalOutput").ap()

    ADD = mybir.AluOpType.add
    SIG = mybir.ActivationFunctionType.Sigmoid
    TANH = mybir.ActivationFunctionType.Tanh

    with ExitStack() as ctx:
        tc = ctx.enter_context(tile.TileContext(nc))
        cp = ctx.enter_context(tc.tile_pool(name="const", bufs=1))
        sp = ctx.enter_context(tc.tile_pool(name="state", bufs=1))
        wp = ctx.enter_context(tc.tile_pool(name="work", bufs=2))
        ps1 = ctx.enter_context(tc.tile_pool(name="ps1", bufs=4, space="PSUM"))
        ps2 = ctx.enter_context(tc.tile_pool(name="ps2", bufs=2, space="PSUM"))

        xts = cp.tile([128, KC, TBS], f16)
        wih_sb = cp.tile([128, KC, GC, 128], f16)
        whh_sb = cp.tile([128, KC, GC, 128], f16)
        bias_sb = cp.tile([128, GC], f32)
        bhhn_sb = cp.tile([128, KC, NB], f32)
        xg_sb = cp.tile([128, GC, TBS], f16)

        nc.sync.dma_start(wih_sb[:], wih_d[:])
        nc.sync.dma_start(whh_sb[:], whh_d[:])
        nc.sync.dma_start(bias_sb[:], biasg_d[:])
        nc.sync.dma_start(bhhn_sb[:], bhhn_d[:])
        nc.sync.dma_start(xts[:], xT[:])

        # 4 fixed h-state slots; step s reads slot s%4, writes (s+1)%4.
        h_slots = [sp.tile([128, KC, NB], f16, tag=f"h{i}", name=f"h{i}")
                   for i in range(4)]
        nc.sync.dma_start(h_slots[0][:], h0t_d[:])

        # ---- Phase 1: xg[g, t*NB+b] = sum_kc wih[kc,g]^T @ x[kc] + bias[g]
        blk = min(P1_BLOCK, TBS)
        for nb in range(TBS // blk):
            lo, hi = nb * blk, (nb + 1) * blk
            for g in range(GC):
                ps = ps1.tile([128, blk], f32, tag="p1")
                for kc in range(KC):
                    nc.tensor.matmul(ps[:], wih_sb[:, kc, g, :], xts[:, kc, lo:hi],
                                     start=(kc == 0), stop=(kc == KC - 1))
                nc.vector.tensor_scalar_add(xg_sb[:, g, lo:hi], ps[:],
                                            bias_sb[:, g:g + 1])

        # ---- Phase 2: the recurrence
        step_idx = [0]

        def body(iv):
            s = step_idx[0]
            h_prev = h_slots[s % 4]
            h_new = h_slots[(s + 1) % 4]
            step_idx[0] += 1

            ps = ps2.tile([128, GC, NB], f32, tag="ps")
            for g in range(GC):
                for kc in range(KC):
                    nc.tensor.matmul(ps[:, g, :], whh_sb[:, kc, g, :],
                                     h_prev[:, kc, :],
                                     start=(kc == 0), stop=(kc == KC - 1))

            xg_r = xg_sb[:, 0:4, bass.ds(iv * NB, NB)]
            xg_z = xg_sb[:, 4:8, bass.ds(iv * NB, NB)]
            xg_n = xg_sb[:, 8:12, bass.ds(iv * NB, NB)]

            rpre = wp.tile([128, KC, NB], f32, tag="rpre")
            nc.vector.tensor_tensor(rpre[:], ps[:, 0:4, :], xg_r, op=ADD)
            r = wp.tile([128, KC, NB], f32, tag="r")
            nc.scalar.activation(r[:], rpre[:], SIG)

            zpre = wp.tile([128, KC, NB], f32, tag="zpre")
            nc.vector.tensor_tensor(zpre[:], ps[:, 4:8, :], xg_z, op=ADD)
            z = wp.tile([128, KC, NB], f32, tag="z")
            nc.scalar.activation(z[:], zpre[:], SIG)
            zp = wp.tile([128, KC, NB], f32, tag="zp")
            nc.scalar.activation(zp[:], zpre[:], SIG, scale=-1.0)

            ghn = wp.tile([128, KC, NB], f32, tag="ghn")
            nc.vector.tensor_tensor(ghn[:], ps[:, 8:12, :], bhhn_sb[:], op=ADD)
            t1 = wp.tile([128, KC, NB], f32, tag="t1")
            nc.vector.tensor_mul(t1[:], r[:], ghn[:])
            npre = wp.tile([128, KC, NB], f32, tag="npre")
            nc.vector.tensor_tensor(npre[:], t1[:], xg_n, op=ADD)
            n = wp.tile([128, KC, NB], f32, tag="n")
            nc.scalar.activation(n[:], npre[:], TANH)

            a = wp.tile([128, KC, NB], f32, tag="a")
            nc.vector.tensor_mul(a[:], h_prev[:], z[:])
            m2 = wp.tile([128, KC, NB], f32, tag="m2")
            nc.vector.tensor_mul(m2[:], zp[:], n[:])
            nc.vector.tensor_tensor(h_new[:], a[:], m2[:], op=ADD)

            nc.sync.dma_start(y_d[bass.ds(iv, 1), :, :, :], h_new[:])

        import concourse.mybir as _mybir

        def unrollable_body(iv0, n_unroll):
            for i in range(n_unroll):
                body(iv0 + i)

        tc.For_i_unrolled_general(0, t_steps, 1, unrollable_body, unroll,
                                  hint_engines=(_mybir.EngineType.PE,))

    nc.compile()
    return nc


def _prep_core(x, h0, w_ih, w_hh, b_ih, b_hh, d, j, t_steps=T):
    bsl = slice(j * NB, (j + 1) * NB)
    xd = x if d == 0 else x[::-1]
    xs = np.ascontiguousarray(xd[:, bsl, :])                     # (T, NB, I)
    xT_ = xs.reshape(t_steps, NB, KC, 128).transpose(3, 2, 0, 1)
    xT_ = np.ascontiguousarray(xT_).reshape(128, KC, t_steps * NB)
    wih = w_ih[d].reshape(GC, 128, KC, 128).transpose(3, 2, 0, 1)
    whh = w_hh[d].reshape(GC, 128, KC, 128).transpose(3, 2, 0, 1)
    bb = b_ih[d].copy()
    bb[:2 * H] += b_hh[d][:2 * H]
    biasg = bb.reshape(GC, 128).T
    bh = b_hh[d][2 * H:].reshape(KC, 128).T                      # (128, KC)
    bhhn = np.repeat(bh[:, :, None], NB, axis=2)                 # (128, KC, NB)
    h0t = h0[d, bsl].reshape(NB, KC, 128).transpose(2, 1, 0)     # (128, KC, NB)
    return {
        "xT": xT_.astype(np.float16),
        "wih": np.ascontiguousarray(wih).astype(np.float16),
        "whh": np.ascontiguousarray(whh).astype(np.float16),
        "biasg": np.ascontiguousarray(biasg).astype(np.float32),
        "bhhn": np.ascontiguousarray(bhhn).astype(np.float32),
        "h0t": np.ascontiguousarray(h0t).astype(np.float16),
    }


def _assemble(y_cores, t_steps=T):
    y_full = np.zeros((t_steps, B, 2 * H), np.float32)
    hn = np.zeros((2, B, H), np.float32)
    for c in range(N_CORES):
        d, j = divmod(c, 4)
        a = y_cores[c].astype(np.float32)            # (T, 128, KC, NB)
        a = a.transpose(0, 3, 2, 1).reshape(t_steps, NB, H)
        hn[d, j * NB:(j + 1) * NB] = a[-1]
        if d == 1:
            a = a[::-1]
        y_full[:, j * NB:(j + 1) * NB, d * H:(d + 1) * H] = a
    return y_full, hn


_cache = {}


def _get_nc():
    if "nc" not in _cache:
        _cache["nc"] = build()
    return _cache["nc"]


def kernel(x, h0, w_ih, w_hh, b_ih, b_hh, _trace=False):
    from concourse import bass_utils

    x = np.asarray(x, np.float32)
    h0 = np.asarray(h0, np.float32)
    w_ih = np.asarray(w_ih, np.float32)
    w_hh = np.asarray(w_hh, np.float32)
    b_ih = np.asarray(b_ih, np.float32)
    b_hh = np.asarray(b_hh, np.float32)

    nc = _get_nc()
    in_maps = []
    for c in range(N_CORES):
        d, j = divmod(c, 4)
        in_maps.append(_prep_core(x, h0, w_ih, w_hh, b_ih, b_hh, d, j))

    res = bass_utils.run_bass_kernel_spmd(nc, in_maps, list(range(N_CORES)),
                                          trace=_trace)
    y_cores = [res.results[c]["y"] for c in range(N_CORES)]
    y_full, hn = _assemble(y_cores)
    if _trace:
        kernel.last_result = res
    return y_full, hn
